# revision 15
# baseline (speedup 1.0000x reference)
"""Causal self-attention (B=4, T=2048, C=1024, H=16, D=64) on 8 trn2 NeuronCores.

Sharding: core g owns (batch b = g//2, head-half hh = g%2), i.e. one batch and
8 heads = 4 head-pairs per core:
  - W_attn columns for those heads' q/k/v (4 pair-groups of 128 each per
    q/k/v) -> per-core [1024, 1536]
  - W_proj rows for those heads' channels -> per-core [512, 1024]
Each core computes a [2048, 1024] partial (its head-half's contribution to its
batch); the host sums the 2 partials per batch (row-parallel W_proj reduce).

Device layout notes (per head-pair, same structure as the head-sharded v1):
  - x_b is passed as X^T [C, T] so every matmul contracts over the partition dim.
  - Attention uses the S^T = K @ Q^T formulation: S^T tiles are [k_tok, q_tok]
    so exp(S)*mask and the P^T @ V matmul need no on-chip transposes of P.
  - The softmax normalizer l[q] = sum_k P[k,q] comes from a ones column
    appended to V (stationary operand): one PSUM accumulation yields [y^T ; l].
  - Normalization multiplies y^T by broadcast(1/l) (K=1 matmul broadcast).
  - All matmul operands are bf16 (PSUM accumulation stays fp32); exp writes
    bf16 P directly from the ScalarE activation.
  - The output projection accumulates all 4 head-pairs into one PSUM tile
    (start/stop over the pair loop), so PSUM->SBUF staging + output DMA is
    4x smaller than per-pair partials; staging copies run on GpSimd to keep
    DVE/ScalarE free for the attention stream.
  - Schedule: QKV projection + V-transposes of token-tile tt+1 and the
    deferred projection of q-tile tt-1 are interleaved with the 4 head-pair
    attention passes of q-tile tt.
"""

import numpy as np

B, T, C, H, D = 4, 2048, 1024, 16, 64
NCORES = 8
HPC = 8                         # heads per core
NPAIR = HPC // 2                # 4 head-pairs per core
CPC = HPC * D                   # 512 channels per core
NC_CHUNKS = C // 128            # 8 contraction chunks of X^T
NGRP = 3 * NPAIR                # 12 qkv output groups of 128
QW = 512                        # q-tile width (moving dim)
KW = 128                        # k-tile width (S^T partition dim)
NTT = T // QW                   # 4 token tiles

_CACHE = {}
LAST_RESULTS = None             # test harness reads exec_time_ns from here


def _build_bass():
    import concourse.bass as bass
    import concourse.mybir as mybir
    import concourse.tile as tile
    from concourse import bacc
    from concourse.masks import make_identity, make_upper_triangular

    f32 = mybir.dt.float32
    bf16 = mybir.dt.bfloat16
    Exp = mybir.ActivationFunctionType.Exp

    nc = bacc.Bacc()
    xt = nc.dram_tensor("xt", [C, T], bf16, kind="ExternalInput")
    wg = nc.dram_tensor("wg", [C, NGRP * 128], bf16, kind="ExternalInput")
    bg = nc.dram_tensor("bg", [NGRP * 128], f32, kind="ExternalInput")
    wp = nc.dram_tensor("wp", [CPC, C], bf16, kind="ExternalInput")
    outp = nc.dram_tensor("outp", [T, C], f32, kind="ExternalOutput")

    with tile.TileContext(nc) as tc:
        with (
            tc.tile_pool(name="const", bufs=1) as cpool,
            tc.tile_pool(name="sb", bufs=2) as sb,
            tc.tile_pool(name="ps", bufs=2, space="PSUM") as ps,
        ):
            # ---- constants ----
            scratch = cpool.tile([128, 128], f32, tag="scratch")
            make_identity(nc, scratch)
            identity = cpool.tile([128, 128], bf16, tag="ident")
            nc.vector.tensor_copy(identity, scratch)
            # mask[k, q] = 1.0 where q >= k else 0 (upper triangular incl diag)
            maskf = cpool.tile([128, 128], f32, tag="maskf")
            make_upper_triangular(nc, maskf, val=1.0, diag=True)
            mask = cpool.tile([128, 128], bf16, tag="mask")
            nc.vector.tensor_copy(mask, maskf)
            # broadcast mask over the head axis (free-dim stride 0)
            mask2 = bass.AP(
                tensor=mask.tensor, offset=mask.offset,
                ap=[mask.ap[0], [0, 2], mask.ap[1]],
            )
            scratch2 = cpool.tile([128, 64], f32, tag="scratch2")
            nc.gpsimd.memset(scratch2, 1.0)
            ones_row = cpool.tile([128, 64], bf16, tag="ones")
            nc.vector.tensor_copy(ones_row, scratch2)

            # ---- weights ----
            wg_sb = []
            for ci in range(NC_CHUNKS):
                wgt = cpool.tile([128, NGRP * 128], bf16, tag=f"wg{ci}")
                nc.sync.dma_start(out=wgt, in_=wg[ci * 128:(ci + 1) * 128, :])
                wg_sb.append(wgt)
            wp_sb = []
            for hp in range(NPAIR):
                wpt = cpool.tile([128, C], bf16, tag=f"wp{hp}")
                nc.sync.dma_start(out=wpt, in_=wp[hp * 128:(hp + 1) * 128, :])
                wp_sb.append(wpt)
            bias_sb = []
            for grp in range(NGRP):
                bt_ = cpool.tile([128, 1], f32, tag=f"bias{grp}")
                nc.sync.dma_start(
                    out=bt_,
                    in_=bg[grp * 128:(grp + 1) * 128].rearrange("(p o) -> p o", o=1),
                )
                bias_sb.append(bt_)

            # per head-pair slabs: [q, k, v][pair] -> [128, T]
            slabs = {}
            for hp in range(NPAIR):
                slabs[hp] = (
                    cpool.tile([128, T], bf16, tag=f"q{hp}", name=f"q{hp}"),
                    cpool.tile([128, T], bf16, tag=f"k{hp}", name=f"k{hp}"),
                    cpool.tile([128, T], bf16, tag=f"v{hp}", name=f"v{hp}"),
                )
            vaug = {hp: [] for hp in range(NPAIR)}  # hp -> [128, 2, 65] tiles
            yts = {}            # (hp, qt) -> yt_sb [128, QW]

            xtiles = {}         # tt -> list of 8 xtiles

            def qkv_load(tt):
                """Start x-tile DMAs for token tile tt (gpsimd queue, so they
                don't serialize behind the weight DMAs on the sync queue)."""
                xts = []
                for ci in range(NC_CHUNKS):
                    xtile = sb.tile([128, QW], bf16, tag="xt", bufs=18, name="xtile")
                    nc.gpsimd.dma_start(
                        out=xtile,
                        in_=xt[ci * 128:(ci + 1) * 128,
                               tt * QW: (tt + 1) * QW],
                    )
                    xts.append(xtile)
                xtiles[tt] = xts

            # group emission order: v first (so vtrans can start early), then
            # k, then q. grp2dest[j] = (kind, hp) for emitted chunk index j.
            GORDER = [(2, 0), (2, 1), (2, 2), (2, 3),
                      (1, 0), (1, 1), (1, 2), (1, 3),
                      (0, 0), (0, 1), (0, 2), (0, 3)]

            # initial tile: q0/k0 first so the first attention pass can start
            # as soon as two bias-adds retire, then v-groups for vtrans(0).
            GORDER0 = [(0, 0), (1, 0), (2, 0), (2, 1), (2, 2), (2, 3),
                       (0, 1), (1, 1), (0, 2), (1, 2), (0, 3), (1, 3)]

            def qkv_groups(tt, j0, j1, order=None):
                """QKV projection groups GORDER[j0:j1] for token tile tt."""
                xts = xtiles[tt]
                for j in range(j0, j1):
                    kind, hp = (order or GORDER)[j]
                    grp = kind * NPAIR + hp
                    dest = slabs[hp][kind]
                    pqkv = ps.tile([128, QW], f32, tag="mm", name="pqkv")
                    for ci in range(NC_CHUNKS):
                        nc.tensor.matmul(
                            pqkv,
                            wg_sb[ci][:, grp * 128:(grp + 1) * 128],
                            xts[ci],
                            start=(ci == 0),
                            stop=(ci == NC_CHUNKS - 1),
                        )
                    nc.vector.tensor_scalar_add(
                        out=dest[:, tt * QW:(tt + 1) * QW],
                        in0=pqkv,
                        scalar1=bias_sb[grp],
                    )

            def vtrans(tt, pairs):
                """V^T -> [V_A | 1 | V_B | 1] tiles for given pairs, k-tiles of tt."""
                for hp in pairs:
                    vt_sb = slabs[hp][2]
                    for kt in range(tt * 4, tt * 4 + 4):
                        ptr = ps.tile([128, 128], bf16, tag="mm",
                                      padded_shape=[128, 512], name="ptr")
                        nc.tensor.transpose(
                            ptr, vt_sb[:, kt * KW:(kt + 1) * KW], identity
                        )
                        va = sb.tile([128, 2, D + 1], bf16, tag="vaug", bufs=68,
                                     name="va")
                        nc.vector.tensor_copy(
                            va[:, :, 0:D],
                            ptr[:, 0:2 * D].rearrange("p (h x) -> p h x", x=D),
                        )
                        nc.vector.tensor_copy(va[:, :, D:D + 1], ones_row[:, 0:2])
                        vaug[hp].append(va)

            def attention_qtile(hp, qt):
                qt_sb, kt_sb, _ = slabs[hp]
                y2 = ps.tile([D + 1, 2, QW], f32, tag="y", bufs=1, name="y2")
                nkt = (qt + 1) * (QW // KW)
                kdiag = qt * (QW // KW)      # first diagonal k-tile
                for kt in range(nkt):
                    diag = kt >= kdiag
                    qoff = (kt - kdiag) * KW if diag else 0
                    w = QW - qoff
                    qsl = slice(qt * QW + qoff, (qt + 1) * QW)
                    ksl = slice(kt * KW, (kt + 1) * KW)
                    st = ps.tile([128, 2, QW], f32, tag="st", name="st")
                    nc.tensor.matmul(
                        st[:, 0, 0:w], kt_sb[0:64, ksl], qt_sb[0:64, qsl]
                    )
                    nc.tensor.matmul(
                        st[:, 1, 0:w], kt_sb[64:128, ksl], qt_sb[64:128, qsl]
                    )
                    p = sb.tile([128, 2, QW], bf16, tag="p", bufs=6, name="p")
                    nc.scalar.activation(
                        p[:, :, 0:w], st[:, :, 0:w], Exp, scale=1.0 / np.sqrt(D)
                    )
                    if diag:
                        # gpsimd, not DVE: keeps the exp->mask->PV chain off
                        # the congested DVE queue (p lives in SBUF, so the
                        # Pool engine may touch it)
                        nc.gpsimd.tensor_mul(
                            p[:, :, 0:KW], p[:, :, 0:KW], mask2
                        )
                    va = vaug[hp][kt]
                    nc.tensor.matmul(
                        y2[:, 0, qoff:QW], va[:, 0, :], p[:, 0, 0:w],
                        start=(kt == 0), stop=(kt == nkt - 1),
                    )
                    nc.tensor.matmul(
                        y2[:, 1, qoff:QW], va[:, 1, :], p[:, 1, 0:w],
                        start=(kt == 0), stop=(kt == nkt - 1),
                    )

                # ---- normalize: y^T * broadcast(1/l) -> yt_sb [128, QW] ----
                ystage = sb.tile([128, 2, QW], f32, tag="ystage", name="ystage")
                nc.vector.tensor_copy(ystage[0:D + 1, :, :], y2[0:D + 1, :, :])
                # 1/l on one DVE lane costs ~6.5 cyc/elem; spread the 1024
                # l-values over 32 partitions with a 32x32 stream-transpose,
                # reciprocal there (32 elems/lane), and transpose back.
                lrow = ystage.rearrange("p h q -> p (h q)")
                lt = sb.tile([128, 2 * QW], f32, tag="lt", name="lt")
                nc.vector.transpose(lt[D:D + 32, :], lrow[D:D + 32, :])
                rt = sb.tile([128, 2 * QW], f32, tag="rt", name="rt")
                lt_v = lt[D:D + 32, :].rearrange("p (j c) -> p j c", c=32)
                rt_v = rt[D:D + 32, :].rearrange("p (j c) -> p j c", c=32)
                nc.vector.reciprocal(rt_v[:, :, 0:1], lt_v[:, :, 0:1])
                rcf = sb.tile([128, 2 * QW], f32, tag="rcf", name="rcf")
                nc.vector.transpose(rcf[D:D + 32, :], rt[D:D + 32, :])
                rcr = sb.tile([128, 2 * QW], bf16, tag="recipr", name="rcr")
                nc.vector.tensor_copy(rcr[D:D + 1, :], rcf[D:D + 1, :])
                bca = ps.tile([64, QW], f32, tag="mm", name="bca")
                bcb = ps.tile([64, QW], f32, tag="mm", name="bcb")
                nc.tensor.matmul(bca, ones_row[D:D + 1, 0:64], rcr[D:D + 1, 0:QW])
                nc.tensor.matmul(bcb, ones_row[D:D + 1, 0:64], rcr[D:D + 1, QW:2 * QW])
                yt_sb = sb.tile([128, QW], bf16, tag="yt", bufs=10, name="yt_sb")
                nc.vector.tensor_mul(yt_sb[0:64, :], ystage[0:D, 0, :], bca)
                nc.vector.tensor_mul(yt_sb[64:128, :], ystage[0:D, 1, :], bcb)
                yts[(hp, qt)] = yt_sb

            def proj_qtile(qt):
                """Output projection for q-tile qt: accumulate all 4 pairs."""
                for m in range(QW // 128):
                    osb = sb.tile([128, C], f32, tag="osb", bufs=3, name="osb")
                    for n in range(C // 512):
                        pp = ps.tile([128, 512], f32, tag="mm", name="pp")
                        for hp in range(NPAIR):
                            nc.tensor.matmul(
                                pp, yts[(hp, qt)][:, m * 128:(m + 1) * 128],
                                wp_sb[hp][:, n * 512:(n + 1) * 512],
                                start=(hp == 0),
                                stop=(hp == NPAIR - 1),
                            )
                        if n == 0:
                            nc.scalar.copy(osb[:, n * 512:(n + 1) * 512], pp)
                        else:
                            nc.vector.tensor_copy(osb[:, n * 512:(n + 1) * 512], pp)
                    row0 = qt * QW + m * 128
                    nc.gpsimd.dma_start(out=outp[row0:row0 + 128, :], in_=osb)

            # ---- software-pipelined schedule ----
            # tt=0 up front; later token-tiles' QKV is split into 3-group
            # chunks emitted after each head-pair's attention so PE-only work
            # fills the exp-wait bubbles evenly. v-groups come first so the
            # V-transposes of tt+1 can follow right after.
            qkv_load(0)
            qkv_groups(0, 0, NGRP, order=GORDER0)
            vtrans(0, range(NPAIR))
            for tt in range(NTT):
                for hp in range(NPAIR):
                    attention_qtile(hp, tt)
                    if tt + 1 < NTT:
                        if hp == 0:
                            qkv_load(tt + 1)
                        qkv_groups(tt + 1, 3 * hp, 3 * (hp + 1))
                        if hp == 1:
                            vtrans(tt + 1, (0, 1, 2))
                        elif hp == 2:
                            vtrans(tt + 1, (3,))
                    if hp == 2 and tt >= 1:
                        proj_qtile(tt - 1)
            proj_qtile(NTT - 1)

    nc.finalize()
    return nc


def _get_nc():
    if "nc" not in _CACHE:
        _CACHE["nc"] = _build_bass()
    return _CACHE["nc"]


def kernel(x, W_attn, b_attn, W_proj, b_proj):
    global LAST_RESULTS
    import ml_dtypes
    from concourse import bass_utils

    bf16 = ml_dtypes.bfloat16
    x = np.asarray(x, dtype=np.float32)
    W_attn = np.asarray(W_attn, dtype=np.float32)
    b_attn = np.asarray(b_attn, dtype=np.float32)
    W_proj = np.asarray(W_proj, dtype=np.float32)
    b_proj = np.asarray(b_proj, dtype=np.float32)

    in_maps = []
    for g in range(NCORES):
        b, hh = divmod(g, 2)
        cols = slice(hh * CPC, (hh + 1) * CPC)
        xt_g = np.ascontiguousarray(x[b].T.astype(bf16))        # [C, T]
        wg_g = np.ascontiguousarray(np.concatenate(
            [W_attn[:, 0 * C:1 * C][:, cols],
             W_attn[:, 1 * C:2 * C][:, cols],
             W_attn[:, 2 * C:3 * C][:, cols]],
            axis=1,
        ).astype(bf16))                                         # [C, 1536]
        bg_g = np.ascontiguousarray(np.concatenate(
            [b_attn[0 * C:1 * C][cols],
             b_attn[1 * C:2 * C][cols],
             b_attn[2 * C:3 * C][cols]]
        ))
        wp_g = np.ascontiguousarray(W_proj[cols, :].astype(bf16))
        in_maps.append({"xt": xt_g, "wg": wg_g, "bg": bg_g, "wp": wp_g})

    nc = _get_nc()
    res = bass_utils.run_bass_kernel_spmd(nc, in_maps, core_ids=list(range(NCORES)))
    LAST_RESULTS = res

    out = np.empty((B, T, C), dtype=np.float32)
    bp = b_proj.astype(np.float64)
    for b in range(B):
        acc = (res.results[2 * b]["outp"].astype(np.float64)
               + res.results[2 * b + 1]["outp"].astype(np.float64) + bp)
        out[b] = acc.astype(np.float32)
    return out


# revision 16
# speedup vs baseline: 1.0265x; 1.0265x over previous
"""Causal self-attention (B=4, T=2048, C=1024, H=16, D=64) on 8 trn2 NeuronCores.

Sharding: core g owns (batch b = g//2, head-half hh = g%2), i.e. one batch and
8 heads = 4 head-pairs per core:
  - W_attn columns for those heads' q/k/v (4 pair-groups of 128 each per
    q/k/v) -> per-core [1024, 1536]
  - W_proj rows for those heads' channels -> per-core [512, 1024]
Each core computes a [2048, 1024] partial (its head-half's contribution to its
batch); the host sums the 2 partials per batch (row-parallel W_proj reduce).

Device layout notes (per head-pair, same structure as the head-sharded v1):
  - x_b is passed as X^T [C, T] so every matmul contracts over the partition dim.
  - Attention uses the S^T = K @ Q^T formulation: S^T tiles are [k_tok, q_tok]
    so exp(S)*mask and the P^T @ V matmul need no on-chip transposes of P.
  - The softmax normalizer l[q] = sum_k P[k,q] comes from a ones column
    appended to V (stationary operand): one PSUM accumulation yields [y^T ; l].
  - Normalization multiplies y^T by broadcast(1/l) (K=1 matmul broadcast).
  - All matmul operands are bf16 (PSUM accumulation stays fp32); exp writes
    bf16 P directly from the ScalarE activation.
  - The output projection accumulates all 4 head-pairs into one PSUM tile
    (start/stop over the pair loop), so PSUM->SBUF staging + output DMA is
    4x smaller than per-pair partials; staging copies run on GpSimd to keep
    DVE/ScalarE free for the attention stream.
  - Schedule: QKV projection + V-transposes of token-tile tt+1 and the
    deferred projection of q-tile tt-1 are interleaved with the 4 head-pair
    attention passes of q-tile tt.
"""

import numpy as np

B, T, C, H, D = 4, 2048, 1024, 16, 64
NCORES = 8
HPC = 8                         # heads per core
NPAIR = HPC // 2                # 4 head-pairs per core
CPC = HPC * D                   # 512 channels per core
NC_CHUNKS = C // 128            # 8 contraction chunks of X^T
NGRP = 3 * NPAIR                # 12 qkv output groups of 128
QW = 512                        # q-tile width (moving dim)
KW = 128                        # k-tile width (S^T partition dim)
NTT = T // QW                   # 4 token tiles

_CACHE = {}
LAST_RESULTS = None             # test harness reads exec_time_ns from here


def _build_bass():
    import concourse.bass as bass
    import concourse.mybir as mybir
    import concourse.tile as tile
    from concourse import bacc
    from concourse.masks import make_identity, make_upper_triangular

    f32 = mybir.dt.float32
    bf16 = mybir.dt.bfloat16
    Exp = mybir.ActivationFunctionType.Exp

    nc = bacc.Bacc()
    xt = nc.dram_tensor("xt", [C, T], bf16, kind="ExternalInput")
    wg = nc.dram_tensor("wg", [C, NGRP * 128], bf16, kind="ExternalInput")
    bg = nc.dram_tensor("bg", [NGRP * 128], f32, kind="ExternalInput")
    wp = nc.dram_tensor("wp", [CPC, C], bf16, kind="ExternalInput")
    outp = nc.dram_tensor("outp", [T, C], f32, kind="ExternalOutput")

    with tile.TileContext(nc) as tc:
        with (
            tc.tile_pool(name="const", bufs=1) as cpool,
            tc.tile_pool(name="sb", bufs=2) as sb,
            tc.tile_pool(name="ps", bufs=2, space="PSUM") as ps,
        ):
            # ---- constants ----
            scratch = cpool.tile([128, 128], f32, tag="scratch")
            make_identity(nc, scratch)
            identity = cpool.tile([128, 128], bf16, tag="ident")
            nc.vector.tensor_copy(identity, scratch)
            # mask[k, q] = 1.0 where q >= k else 0 (upper triangular incl diag)
            maskf = cpool.tile([128, 128], f32, tag="maskf")
            make_upper_triangular(nc, maskf, val=1.0, diag=True)
            mask = cpool.tile([128, 128], bf16, tag="mask")
            nc.vector.tensor_copy(mask, maskf)
            # broadcast mask over the head axis (free-dim stride 0)
            mask2 = bass.AP(
                tensor=mask.tensor, offset=mask.offset,
                ap=[mask.ap[0], [0, 2], mask.ap[1]],
            )
            scratch2 = cpool.tile([128, 64], f32, tag="scratch2")
            nc.gpsimd.memset(scratch2, 1.0)
            ones_row = cpool.tile([128, 64], bf16, tag="ones")
            nc.vector.tensor_copy(ones_row, scratch2)

            # ---- weights ----
            wg_sb = []
            for ci in range(NC_CHUNKS):
                wgt = cpool.tile([128, NGRP * 128], bf16, tag=f"wg{ci}")
                nc.sync.dma_start(out=wgt, in_=wg[ci * 128:(ci + 1) * 128, :])
                wg_sb.append(wgt)
            wp_sb = []
            for hp in range(NPAIR):
                wpt = cpool.tile([128, C], bf16, tag=f"wp{hp}")
                nc.sync.dma_start(out=wpt, in_=wp[hp * 128:(hp + 1) * 128, :])
                wp_sb.append(wpt)
            bias_sb = []
            for grp in range(NGRP):
                bt_ = cpool.tile([128, 1], f32, tag=f"bias{grp}")
                nc.sync.dma_start(
                    out=bt_,
                    in_=bg[grp * 128:(grp + 1) * 128].rearrange("(p o) -> p o", o=1),
                )
                bias_sb.append(bt_)

            # per head-pair slabs: [q, k, v][pair] -> [128, T]
            slabs = {}
            for hp in range(NPAIR):
                slabs[hp] = (
                    cpool.tile([128, T], bf16, tag=f"q{hp}", name=f"q{hp}"),
                    cpool.tile([128, T], bf16, tag=f"k{hp}", name=f"k{hp}"),
                    cpool.tile([128, T], bf16, tag=f"v{hp}", name=f"v{hp}"),
                )
            vaug = {hp: [] for hp in range(NPAIR)}  # hp -> [128, 2, 65] tiles
            yts = {}            # (hp, qt) -> yt_sb [128, QW]

            xtiles = {}         # tt -> list of 8 xtiles

            def qkv_load(tt):
                """Start x-tile DMAs for token tile tt (gpsimd queue, so they
                don't serialize behind the weight DMAs on the sync queue)."""
                xts = []
                for ci in range(NC_CHUNKS):
                    xtile = sb.tile([128, QW], bf16, tag="xt", bufs=18, name="xtile")
                    nc.gpsimd.dma_start(
                        out=xtile,
                        in_=xt[ci * 128:(ci + 1) * 128,
                               tt * QW: (tt + 1) * QW],
                    )
                    xts.append(xtile)
                xtiles[tt] = xts

            # group emission order: v first (so vtrans can start early), then
            # k, then q. grp2dest[j] = (kind, hp) for emitted chunk index j.
            GORDER = [(2, 0), (2, 1), (2, 2), (2, 3),
                      (1, 0), (1, 1), (1, 2), (1, 3),
                      (0, 0), (0, 1), (0, 2), (0, 3)]

            # initial tile: q0/k0 first so the first attention pass can start
            # as soon as two bias-adds retire, then v-groups for vtrans(0).
            GORDER0 = [(0, 0), (1, 0), (2, 0), (2, 1), (2, 2), (2, 3),
                       (0, 1), (1, 1), (0, 2), (1, 2), (0, 3), (1, 3)]

            def qkv_groups(tt, j0, j1, order=None):
                """QKV projection groups GORDER[j0:j1] for token tile tt."""
                xts = xtiles[tt]
                for j in range(j0, j1):
                    kind, hp = (order or GORDER)[j]
                    grp = kind * NPAIR + hp
                    dest = slabs[hp][kind]
                    pqkv = ps.tile([128, QW], f32, tag="mm", name="pqkv")
                    for ci in range(NC_CHUNKS):
                        nc.tensor.matmul(
                            pqkv,
                            wg_sb[ci][:, grp * 128:(grp + 1) * 128],
                            xts[ci],
                            start=(ci == 0),
                            stop=(ci == NC_CHUNKS - 1),
                        )
                    nc.vector.tensor_scalar_add(
                        out=dest[:, tt * QW:(tt + 1) * QW],
                        in0=pqkv,
                        scalar1=bias_sb[grp],
                    )

            def vtrans(tt, pairs):
                """V^T -> [V_A | 1 | V_B | 1] tiles for given pairs, k-tiles of tt."""
                for hp in pairs:
                    vt_sb = slabs[hp][2]
                    for kt in range(tt * 4, tt * 4 + 4):
                        ptr = ps.tile([128, 128], bf16, tag="mm",
                                      padded_shape=[128, 512], name="ptr")
                        nc.tensor.transpose(
                            ptr, vt_sb[:, kt * KW:(kt + 1) * KW], identity
                        )
                        va = sb.tile([128, 2, D + 1], bf16, tag="vaug", bufs=68,
                                     name="va")
                        nc.vector.tensor_copy(
                            va[:, :, 0:D],
                            ptr[:, 0:2 * D].rearrange("p (h x) -> p h x", x=D),
                        )
                        nc.vector.tensor_copy(va[:, :, D:D + 1], ones_row[:, 0:2])
                        vaug[hp].append(va)

            def attention_qtile(hp, qt):
                qt_sb, kt_sb, _ = slabs[hp]
                y2 = ps.tile([D + 1, 2, QW], f32, tag="y", bufs=1, name="y2")
                nkt = (qt + 1) * (QW // KW)
                kdiag = qt * (QW // KW)      # first diagonal k-tile
                for kt in range(nkt):
                    diag = kt >= kdiag
                    qoff = (kt - kdiag) * KW if diag else 0
                    w = QW - qoff
                    qsl = slice(qt * QW + qoff, (qt + 1) * QW)
                    ksl = slice(kt * KW, (kt + 1) * KW)
                    st = ps.tile([128, 2, QW], f32, tag="st", name="st")
                    nc.tensor.matmul(
                        st[:, 0, 0:w], kt_sb[0:64, ksl], qt_sb[0:64, qsl]
                    )
                    nc.tensor.matmul(
                        st[:, 1, 0:w], kt_sb[64:128, ksl], qt_sb[64:128, qsl]
                    )
                    p = sb.tile([128, 2, QW], bf16, tag="p", bufs=6, name="p")
                    nc.scalar.activation(
                        p[:, :, 0:w], st[:, :, 0:w], Exp, scale=1.0 / np.sqrt(D)
                    )
                    if diag:
                        nc.vector.tensor_mul(
                            p[:, :, 0:KW], p[:, :, 0:KW], mask2
                        )
                    va = vaug[hp][kt]
                    nc.tensor.matmul(
                        y2[:, 0, qoff:QW], va[:, 0, :], p[:, 0, 0:w],
                        start=(kt == 0), stop=(kt == nkt - 1),
                    )
                    nc.tensor.matmul(
                        y2[:, 1, qoff:QW], va[:, 1, :], p[:, 1, 0:w],
                        start=(kt == 0), stop=(kt == nkt - 1),
                    )

                # ---- normalize: y^T * broadcast(1/l) -> yt_sb [128, QW] ----
                ystage = sb.tile([128, 2, QW], f32, tag="ystage", name="ystage")
                nc.vector.tensor_copy(ystage[0:D + 1, :, :], y2[0:D + 1, :, :])
                # 1/l on one DVE lane costs ~6.5 cyc/elem; spread the 1024
                # l-values over 32 partitions with a 32x32 stream-transpose,
                # reciprocal there (32 elems/lane), and transpose back.
                lrow = ystage.rearrange("p h q -> p (h q)")
                lt = sb.tile([128, 2 * QW], f32, tag="lt", name="lt")
                nc.vector.transpose(lt[D:D + 32, :], lrow[D:D + 32, :])
                rt = sb.tile([128, 2 * QW], f32, tag="rt", name="rt")
                lt_v = lt[D:D + 32, :].rearrange("p (j c) -> p j c", c=32)
                rt_v = rt[D:D + 32, :].rearrange("p (j c) -> p j c", c=32)
                nc.vector.reciprocal(rt_v[:, :, 0:1], lt_v[:, :, 0:1])
                rcf = sb.tile([128, 2 * QW], f32, tag="rcf", name="rcf")
                nc.vector.transpose(rcf[D:D + 32, :], rt[D:D + 32, :])
                rcr = sb.tile([128, 2 * QW], bf16, tag="recipr", name="rcr")
                nc.vector.tensor_copy(rcr[D:D + 1, :], rcf[D:D + 1, :])
                bca = ps.tile([64, QW], f32, tag="mm", name="bca")
                bcb = ps.tile([64, QW], f32, tag="mm", name="bcb")
                nc.tensor.matmul(bca, ones_row[D:D + 1, 0:64], rcr[D:D + 1, 0:QW])
                nc.tensor.matmul(bcb, ones_row[D:D + 1, 0:64], rcr[D:D + 1, QW:2 * QW])
                yt_sb = sb.tile([128, QW], bf16, tag="yt", bufs=10, name="yt_sb")
                nc.vector.tensor_mul(yt_sb[0:64, :], ystage[0:D, 0, :], bca)
                nc.vector.tensor_mul(yt_sb[64:128, :], ystage[0:D, 1, :], bcb)
                yts[(hp, qt)] = yt_sb

            def proj_qtile(qt):
                """Output projection for q-tile qt: accumulate all 4 pairs."""
                for m in range(QW // 128):
                    osb = sb.tile([128, C], f32, tag="osb", bufs=3, name="osb")
                    for n in range(C // 512):
                        pp = ps.tile([128, 512], f32, tag="mm", name="pp")
                        for hp in range(NPAIR):
                            nc.tensor.matmul(
                                pp, yts[(hp, qt)][:, m * 128:(m + 1) * 128],
                                wp_sb[hp][:, n * 512:(n + 1) * 512],
                                start=(hp == 0),
                                stop=(hp == NPAIR - 1),
                            )
                        if n == 0:
                            nc.scalar.copy(osb[:, n * 512:(n + 1) * 512], pp)
                        else:
                            nc.vector.tensor_copy(osb[:, n * 512:(n + 1) * 512], pp)
                    row0 = qt * QW + m * 128
                    nc.gpsimd.dma_start(out=outp[row0:row0 + 128, :], in_=osb)

            # ---- software-pipelined schedule ----
            # (x-tile DMAs ride the gpsimd queue so they never serialize
            # behind the 3MB weight DMA on the sync queue)
            qkv_load(0)
            qkv_groups(0, 0, NGRP, order=GORDER0)
            vtrans(0, range(NPAIR))
            for tt in range(NTT):
                for hp in range(NPAIR):
                    attention_qtile(hp, tt)
                    if hp == 0 and tt + 1 < NTT:
                        qkv_load(tt + 1)
                        qkv_groups(tt + 1, 0, NGRP)
                    if hp == 1 and tt + 1 < NTT:
                        vtrans(tt + 1, range(NPAIR))
                    if hp == 2 and tt >= 1:
                        proj_qtile(tt - 1)
            proj_qtile(NTT - 1)

    nc.finalize()
    return nc


def _get_nc():
    if "nc" not in _CACHE:
        _CACHE["nc"] = _build_bass()
    return _CACHE["nc"]


def kernel(x, W_attn, b_attn, W_proj, b_proj):
    global LAST_RESULTS
    import ml_dtypes
    from concourse import bass_utils

    bf16 = ml_dtypes.bfloat16
    x = np.asarray(x, dtype=np.float32)
    W_attn = np.asarray(W_attn, dtype=np.float32)
    b_attn = np.asarray(b_attn, dtype=np.float32)
    W_proj = np.asarray(W_proj, dtype=np.float32)
    b_proj = np.asarray(b_proj, dtype=np.float32)

    in_maps = []
    for g in range(NCORES):
        b, hh = divmod(g, 2)
        cols = slice(hh * CPC, (hh + 1) * CPC)
        xt_g = np.ascontiguousarray(x[b].T.astype(bf16))        # [C, T]
        wg_g = np.ascontiguousarray(np.concatenate(
            [W_attn[:, 0 * C:1 * C][:, cols],
             W_attn[:, 1 * C:2 * C][:, cols],
             W_attn[:, 2 * C:3 * C][:, cols]],
            axis=1,
        ).astype(bf16))                                         # [C, 1536]
        bg_g = np.ascontiguousarray(np.concatenate(
            [b_attn[0 * C:1 * C][cols],
             b_attn[1 * C:2 * C][cols],
             b_attn[2 * C:3 * C][cols]]
        ))
        wp_g = np.ascontiguousarray(W_proj[cols, :].astype(bf16))
        in_maps.append({"xt": xt_g, "wg": wg_g, "bg": bg_g, "wp": wp_g})

    nc = _get_nc()
    res = bass_utils.run_bass_kernel_spmd(nc, in_maps, core_ids=list(range(NCORES)))
    LAST_RESULTS = res

    out = np.empty((B, T, C), dtype=np.float32)
    bp = b_proj.astype(np.float64)
    for b in range(B):
        acc = (res.results[2 * b]["outp"].astype(np.float64)
               + res.results[2 * b + 1]["outp"].astype(np.float64) + bp)
        out[b] = acc.astype(np.float32)
    return out
